# revision 24
# baseline (speedup 1.0000x reference)
"""Trainium2 Bass kernel for the quirky multi-head attention problem.

Math (per batch b, head a), faithful to the reference:
    K = x[b] @ W_K[a].T          # [S, H]
    Q = x[b] @ W_Q[a].T
    V = x[b] @ W_V[a].T
    s[c, C] = (K @ Q.T)[c, C] / sqrt(H)        rows c = "key" index
    valid iff C <= c (tril); softmax over C per row c
    E = exp(s) * tril            # no max-subtraction: |s| <= ~7, fp32-safe
    denom[c] = sum_C E[c, C]
    z[C, h] = sum_c E[c, C] * (V/denom)[c, h]  # = E.T @ (V/denom)
    out[b] += z @ W_O[a].T

Sharding: 8 cores = 2 batches x 4 head-pairs. Each core handles one batch
and two heads; the attention matrix is device-local. Host sums the four
head-pair partial outputs per batch.

Device layouts (per core):
    xT16  [E, S]      x[b] transposed (host-side), fp16 (all projections)
    wk/wq/wv16 [E, 128]  W[a0].T | W[a1].T concat on head axis, fp16
    wo0/wo1 [128, E]  W_O[a].T duplicated on both partition halves
    maskb [128, 128]  additive causal triangle (0 or -1e9)
    maskm [128, 128]  multiplicative fp16 0/1 triangle (single-wave path)
    outT  [E, S]      partial output, transposed

On-chip flow per head: scores [c_blk=128 rows, 512-wide C chunks] are
matmul'd into rotating PSUM wave tiles (2 banks x 2 bufs); the diagonal
gets an additive -1e9 triangle mask; ScalarE applies exp (scale=1/sqrt(H))
writing the row panel to SBUF (fp16) with a fused per-row accumulation
(softmax denominator). z^T accumulates in PSUM across row blocks; chunk j
of C lives at partition half (j < NCH/2 ? 0 : 64) so z^T fits in 4 banks
and coexists with the wave tiles.

PE scheduling: the exp chain (ACT ~1.1us per 1024-wide wave) is slower
than the PE refill of a wave (~0.5us), so raw score streaming stalls PE
on PSUM recycling.  To keep PE dense, z-stage matmuls are interleaved
at CHUNK granularity: a global FIFO of ready z chunks (populated Z_LAG
row blocks late, once a block's denominator is known) is drained ZPW
chunks after every score/projection wave.  Head 1 sweeps rows in
reverse so z banks finalize early; bank-final events trigger the zsb
copy + output projection inline.  Each head's zT bank-zeroing matmuls
(start=True full-width zero writes, letting every real z chunk
accumulate start=False at its TRUE width — no padded columns) are
deferred to the head's first real z chunk so the head boundary
(head-0 drain -> zsb copy -> zero) rides under head-1's score waves
instead of stalling PE.  PSUM->SBUF copies stay on DVE (GPSIMD cannot
access PSUM on TRN2; and DVE is in-order, so long panel reduces must
NOT be queued there — they stall the latency-critical mask-adds).

NOTE on the clock: the HW power governor (HAM, k-of-n clock duty)
throttles the PE to half clock 55-67% of the time under this sustained
load, and the share drifts +-10pp run to run (~+-25us span).  Dense
scheduling does not unlock clock — only genuine MAC/energy reduction
moves the floor.  fp8 DoubleRow measured SLOWER than fp16 here and
fails accuracy; don't revisit.

Matmul dtypes: fp16 operands everywhere (PSUM accumulates fp32).  The
HAM clock governor throttles the PE under sustained dense activity, so
total PE column-cycles and operand power (not idle-gap avoidance) set
the floor; fp16 halves operand traffic vs f32r and keeps every matmul
at 1 col/cycle (f32r at 128-wide output runs at 1/4 rate).  Score
rounding from fp16 K/Q is ~1e-3 absolute on the exponent -> ~0.1% on
softmax weights.
"""

import math

import numpy as np

B, S_FULL, E, A, H = 2, 4096, 512, 8, 64
N_CORES = 8
NEG_BIG = -1.0e9

import os as _os

Z_LAG = int(_os.environ.get("Z_LAG", "3"))
ZPW = int(_os.environ.get("ZPW", "2"))
PANEL_BUFS = int(_os.environ.get("PANEL_BUFS", "4"))

_prog_cache = {}


def _build_program(S):
    import concourse.mybir as mybir
    import concourse.tile as tile
    from concourse import bacc

    f32 = mybir.dt.float32
    f32r = mybir.dt.float32r
    bf16 = mybir.dt.bfloat16
    fp16 = mybir.dt.float16

    EC = E // 128            # e chunks (contraction for projections)
    NCB = S // 128           # row blocks
    NCH = S // 512           # C chunks per full row
    HALF = NCH // 2          # chunks per partition half of z^T
    assert NCH % 2 == 0

    nc = bacc.Bacc("TRN2", target_bir_lowering=False, debug=False)

    xT16 = nc.dram_tensor("xT16", [E, S], fp16, kind="ExternalInput")
    # all small operands in one partition-major blob: 128 rows x
    # [wk(512) | wq(512) | wv(512) | wo01(512) | maskm(128)] fp16 --
    # 4 DMA transfers with 1-2KB-per-partition descriptors instead of
    # ~1500 256-byte ones that stall the x stream behind them.
    blob = nc.dram_tensor("blob", [128, 2176], fp16, kind="ExternalInput")
    maskb = nc.dram_tensor("maskb", [128, 128], f32, kind="ExternalInput")
    outT = nc.dram_tensor("outT", [E, S], fp16, kind="ExternalOutput")

    ExpF = mybir.ActivationFunctionType.Exp
    AxX = mybir.AxisListType.X
    AluAdd = mybir.AluOpType.add

    with tile.TileContext(nc) as tc:
        with (
            tc.tile_pool(name="singles", bufs=1) as singles,
            tc.tile_pool(name="panelp", bufs=PANEL_BUFS) as panelp,
            tc.tile_pool(name="small", bufs=8) as small,
            tc.tile_pool(name="outst", bufs=4) as outst,
            tc.tile_pool(name="ps", bufs=2, space="PSUM") as ps,
            tc.tile_pool(name="zps", bufs=1, space="PSUM") as zps,
        ):
            # ---- load inputs (small tensors first: weights + masks are on
            # early dependency chains; the bulk x stream follows) ----
            xt16 = singles.tile([128, EC, S], fp16)
            blb = singles.tile([128, 2176], fp16)
            wks = blb[:, 0:512]
            wqs = blb[:, 512:1024]
            wvs = blb[:, 1024:1536]
            wos01 = blb[:, 1536:2048]
            mskm = blb[:, 2048:2176]
            # z^T staging, packed for a single 128-contraction output
            # projection: partitions 0:64 = head 0, 64:128 = head 1;
            # zsbA holds chunk parity 0 (ccn < HALF), zsbB parity 1.
            zsbA = singles.tile([128, HALF * 512], fp16)
            zsbB = singles.tile([128, HALF * 512], fp16)
            msk = singles.tile([128, 128], f32)
            for bq in range(4):
                bsl = slice(bq * 544, (bq + 1) * 544)
                nc.sync.dma_start(out=blb[:, bsl], in_=blob[:, bsl])
            nc.sync.dma_start(out=msk, in_=maskb[:, :])
            SQ = S // 4
            for q in range(4):
                qsl = slice(q * SQ, (q + 1) * SQ)
                for ec in range(EC):
                    nc.sync.dma_start(
                        out=xt16[:, ec, qsl], in_=xT16[ec * 128:(ec + 1) * 128, qsl]
                    )
            # bf16 always: the K=1 zeroing matmuls are invalid ISA in f32r,
            # and mixing dtypes across an accumulation group is fine.
            zero_t = singles.tile([1, 576], bf16)
            nc.vector.memset(zero_t, 0.0)

            kt = singles.tile([128, S], fp16)
            qt = singles.tile([128, S], fp16)
            vsb = singles.tile([128, NCB, 128], fp16)
            CPQ = NCH // 4           # 512-chunks per x quarter

            # ---- global z-chunk FIFO -------------------------------------
            # Items: (ensure_pre, zT, vt, panel, j, poff, col, start, stop,
            #         post) — ensure_pre emits the head's deferred zero
            # matmuls; post (on an item's final chunk) runs bank-final
            # hooks (zsb copy + output projection).
            zq = []

            def pump(k):
                for _ in range(k):
                    if not zq:
                        return
                    ensure, zT_, vt_, panel_, j, poff, col, w_n, sp_, post = zq.pop(0)
                    if ensure is not None:
                        ensure()
                    if vt_ is not None:
                        nc.tensor.matmul(
                            zT_[poff:poff + 64, col:col + w_n],
                            vt_,
                            panel_[:, j * 512:j * 512 + w_n],
                            start=False, stop=sp_,
                            skip_group_check=True,
                        )
                    if post is not None:
                        post()

            # ---- projections (emitted per x-quarter, interleaved into the
            # head-0 narrow blocks so PE stays dense while DMA streams in) ----
            def emit_proj(q):
                for dst, w in ((kt, wks), (qt, wqs)):
                    for cc in range(q * CPQ, (q + 1) * CPQ):
                        csl = slice(cc * 512, (cc + 1) * 512)
                        wt = ps.tile([128, 1024], f32, tag="wave", name="wt")
                        for ec in range(EC):
                            nc.tensor.matmul(
                                wt[:, :512], w[:, ec * 128:(ec + 1) * 128],
                                xt16[:, ec, csl],
                                start=(ec == 0), stop=(ec == EC - 1),
                            )
                        nc.vector.tensor_copy(dst[:, csl], wt[:, :512])
                        pump(ZPW)
                for cb in range(q * (NCB // 4), (q + 1) * (NCB // 4)):
                    bsl = slice(cb * 128, (cb + 1) * 128)
                    wt = ps.tile([128, 1024], f32, tag="wave", name="wt")
                    for ec in range(EC):
                        nc.tensor.matmul(
                            wt[:, :128], xt16[:, ec, bsl],
                            wvs[:, ec * 128:(ec + 1) * 128],
                            start=(ec == 0), stop=(ec == EC - 1),
                        )
                    nc.vector.tensor_copy(vsb[:, cb, :], wt[:, :128])
                    pump(1)

            # ---- attention per head ----
            for h in range(2):
                hs = slice(h * 64, (h + 1) * 64)
                # head 0 forward, head 1 reverse: the reverse head leads with
                # its widest blocks and finalizes z banks early
                order = list(range(NCB - 1, -1, -1)) if h == 1 else list(range(NCB))
                zT = zps.tile([128, HALF * 512], f32, name="zT")

                zero_done = [False]

                def ensure_zero(zT=zT, zero_done=zero_done):
                    # Dummy matmuls zero-start BOTH partition halves of each
                    # z bank; every real z chunk then accumulates with
                    # start=False at its true width (no 512-padding MACs for
                    # diagonal chunks).  Deferred to the head's first z chunk
                    # so the head boundary doesn't stall PE.
                    if not zero_done[0]:
                        zero_done[0] = True
                        for k in range(HALF):
                            for poff in (0, 64):
                                nc.tensor.matmul(
                                    zT[poff:poff + 64, k * 512:(k + 1) * 512],
                                    zero_t[:, :64], zero_t[:, 64:576],
                                    start=True, stop=False,
                                    skip_group_check=True,
                                )

                first_cb = {}
                last_cb = {}
                for j in range(NCH):
                    part = [cb for cb in order if cb >= 4 * j]
                    first_cb[j] = part[0]
                    last_cb[j] = part[-1]

                def emit_out_proj(ccn):
                    zsb = zsbA if ccn < HALF else zsbB
                    col = (ccn % HALF) * 512
                    for ecn in range(EC):
                        esl = slice(ecn * 128, (ecn + 1) * 128)
                        wt = ps.tile([128, 1024], f32, tag="wave", name="wt")
                        nc.tensor.matmul(
                            wt[:, :512],
                            wos01[:, esl],
                            zsb[:, col:col + 512],
                            start=True, stop=True,
                        )
                        st = outst.tile([128, 512], fp16, name="st")
                        nc.vector.tensor_copy(st, wt[:, :512])
                        nc.sync.dma_start(
                            out=outT[esl, ccn * 512:(ccn + 1) * 512], in_=st
                        )

                def push_z(item, h=h, zT=zT, ensure_zero=ensure_zero,
                           first_cb=first_cb, last_cb=last_cb):
                    vt_i, panel_i, nch_i, cb_i = item
                    cw = cb_i * 128 + 128   # true row length of this block
                    post = None
                    if h == 1 and cb_i % 4 == 0 and cb_i // 4 < HALF:
                        # Reverse sweep: bank q (chunks q, q+HALF) is final
                        # once cb=4q is done; copy it out and trickle the
                        # output projection into the PE stream right here.
                        q = cb_i // 4

                        def post(q=q, zT=zT, h=h):
                            csl = slice(q * 512, (q + 1) * 512)
                            nc.vector.tensor_copy(
                                zsbA[64 * h:64 * h + 64, csl], zT[0:64, csl]
                            )
                            nc.vector.tensor_copy(
                                zsbB[64 * h:64 * h + 64, csl], zT[64:128, csl]
                            )
                            emit_out_proj(q)
                            emit_out_proj(q + HALF)

                    for j in range(nch_i):
                        poff = 0 if j < HALF else 64
                        col = (j % HALF) * 512
                        w_n = min(cw - j * 512, 512)
                        stop = cb_i == last_cb[j]
                        zq.append((
                            ensure_zero, zT, vt_i, panel_i, j, poff, col,
                            w_n, stop,
                            post if j == nch_i - 1 else None,
                        ))

                pending = []
                GRP = NCB // 4
                for oi, cb in enumerate(order):
                    if h == 0 and oi % GRP == 0:
                        emit_proj(oi // GRP)
                    c0 = cb * 128
                    nch = (c0 + 128 + 511) // 512
                    nwaves = (nch + 1) // 2
                    lastw = c0 + 128 - (nch - 1) * 512   # width of diag chunk
                    panel = panelp.tile([128, S], fp16, name="panel")
                    rsp = small.tile([128, 4], f32, name="rsp")
                    # Multi-wave blocks keep the ACT fused accumulation for
                    # the denominator: DVE is in-order and carries the
                    # latency-critical PSUM mask-adds + wave copies, so
                    # multi-microsecond panel reduces there stall the PSUM
                    # recycle chain (cost ~25us when tried).
                    red_eng = None if nwaves == 1 else "act"
                    for wv_i in range(nwaves):
                        jlo = 2 * wv_i
                        jhi = min(jlo + 2, nch)
                        wt = ps.tile([128, 1024], f32, tag="wave", name="wt")
                        for j in range(jlo, jhi):
                            w_n = lastw if j == nch - 1 else 512
                            nc.tensor.matmul(
                                wt[:, (j - jlo) * 512:(j - jlo) * 512 + w_n],
                                kt[hs, c0:c0 + 128],
                                qt[hs, j * 512:j * 512 + w_n],
                                start=True, stop=True,
                            )
                        if jhi == nch and nwaves > 1:
                            # mask only the last 128 cols (the true triangle);
                            # earlier diag-chunk cols are fully valid
                            o = c0 - (nch - 1) * 512
                            dlo = (nch - 1 - jlo) * 512 + o
                            nc.vector.tensor_add(
                                wt[:, dlo:dlo + 128], wt[:, dlo:dlo + 128],
                                msk,
                            )
                        wlen = (jhi - jlo - 1) * 512 + (lastw if jhi == nch else 512)
                        nc.scalar.activation(
                            out=panel[:, jlo * 512:jlo * 512 + wlen],
                            in_=wt[:, :wlen],
                            func=ExpF,
                            scale=1.0 / math.sqrt(H),
                            accum_out=(rsp[:, wv_i:wv_i + 1]
                                       if red_eng == "act" else None),
                        )
                        pump(ZPW)
                    den = small.tile([128, 1], f32, name="den")
                    if red_eng == "act":
                        nc.vector.tensor_reduce(den, rsp[:, :nwaves], axis=AxX, op=AluAdd)
                    else:
                        # single-wave: mask applied post-exp (0/1 triangle on
                        # the fp16 panel) so exp never waits the mask; the
                        # reduce then sums the masked zeros
                        dpan = (nch - 1) * 512 + (c0 - (nch - 1) * 512)
                        nc.vector.tensor_mul(
                            panel[:, dpan:dpan + 128],
                            panel[:, dpan:dpan + 128], mskm,
                        )
                        nc.vector.tensor_reduce(
                            den, panel[:, :c0 + 128], axis=AxX, op=AluAdd
                        )
                    rden = small.tile([128, 1], f32, name="rden")
                    nc.vector.reciprocal(rden, den)
                    vt = small.tile([128, 64], fp16, name="vt")
                    nc.vector.tensor_scalar_mul(vt, vsb[:, cb, hs], rden)
                    pending.append((vt, panel, nch, cb))
                    if len(pending) > Z_LAG:
                        push_z(pending.pop(0))
                # Spill this head's remaining blocks into the FIFO; head 0's
                # tail rides under head 1's first (widest) score waves.
                for item in pending:
                    push_z(item)
                if h == 0:
                    def h0_copies(zT=zT):
                        for q in range(HALF):
                            csl = slice(q * 512, (q + 1) * 512)
                            nc.vector.tensor_copy(
                                zsbA[0:64, csl], zT[0:64, csl]
                            )
                            nc.vector.tensor_copy(
                                zsbB[0:64, csl], zT[64:128, csl]
                            )
                    # sentinel: runs after head 0's last z chunk pops
                    zq.append((None, None, None, None, 0, 0, 0, 0, False,
                               h0_copies))
            # drain
            pump(len(zq) + 8)

    nc.compile()
    return nc


def get_program(S=S_FULL):
    if S not in _prog_cache:
        _prog_cache[S] = _build_program(S)
    return _prog_cache[S]


def make_mask_band():
    """Triangle mask for the last 128 cols of a diagonal chunk:
    col t (relative to the diagonal start) is valid iff t <= r."""
    r = np.arange(128)[:, None]
    t = np.arange(128)[None, :]
    return np.where(t <= r, 0.0, NEG_BIG).astype(np.float32)


def make_core_inputs(x, W_K, W_Q, W_V, W_O, core):
    """Inputs for core = b*4 + g (batch b, head pair a0=2g, a1=2g+1)."""
    b, g = divmod(core, 4)
    a0, a1 = 2 * g, 2 * g + 1
    xT = np.ascontiguousarray(x[b].T)
    wk = np.ascontiguousarray(np.concatenate([W_K[a0].T, W_K[a1].T], axis=1))
    wq = np.ascontiguousarray(np.concatenate([W_Q[a0].T, W_Q[a1].T], axis=1))
    wv = np.ascontiguousarray(np.concatenate([W_V[a0].T, W_V[a1].T], axis=1))
    wo01 = np.ascontiguousarray(np.concatenate([W_O[a0].T, W_O[a1].T], axis=0))
    tri = make_mask_band()
    f16 = np.float16

    def fold(w):
        # [E, 128] -> [128, EC*128]: SBUF partition p holds w[ec*128+p, :]
        # at cols ec*128..ec*128+128
        return w.reshape(4, 128, 128).transpose(1, 0, 2).reshape(128, 512)

    blob = np.hstack([
        fold(wk), fold(wq), fold(wv), wo01, (tri == 0.0).astype(np.float32),
    ]).astype(f16)
    return {
        "xT16": xT.astype(f16),
        "blob": np.ascontiguousarray(blob),
        "maskb": tri,
    }


def run_on_cores(inputs, trace=False):
    from concourse.bass_utils import run_bass_kernel_spmd

    nc = get_program()
    in_maps = [
        make_core_inputs(
            inputs["x"], inputs["W_K"], inputs["W_Q"], inputs["W_V"],
            inputs["W_O"], core,
        )
        for core in range(N_CORES)
    ]
    return run_bass_kernel_spmd(
        nc, in_maps, list(range(N_CORES)), trace=trace,
    )


def kernel(x, W_K, W_Q, W_V, W_O):
    x = np.asarray(x, dtype=np.float32)
    W_K = np.asarray(W_K, dtype=np.float32)
    W_Q = np.asarray(W_Q, dtype=np.float32)
    W_V = np.asarray(W_V, dtype=np.float32)
    W_O = np.asarray(W_O, dtype=np.float32)
    res = run_on_cores(
        {"x": x, "W_K": W_K, "W_Q": W_Q, "W_V": W_V, "W_O": W_O}
    )
    out = np.zeros((B, S_FULL, E), dtype=np.float32)
    for b in range(B):
        acc = np.zeros((E, S_FULL), dtype=np.float32)
        for g in range(4):
            acc += res.results[b * 4 + g]["outT"]
        out[b] = acc.T
    return out


# revision 25
# speedup vs baseline: 1.1889x; 1.1889x over previous
"""Trainium2 Bass kernel for the quirky multi-head attention problem.

Math (per batch b, head a), faithful to the reference:
    K = x[b] @ W_K[a].T          # [S, H]
    Q = x[b] @ W_Q[a].T
    V = x[b] @ W_V[a].T
    s[c, C] = (K @ Q.T)[c, C] / sqrt(H)        rows c = "key" index
    valid iff C <= c (tril); softmax over C per row c
    E = exp(s) * tril            # no max-subtraction: |s| <= ~7, fp32-safe
    denom[c] = sum_C E[c, C]
    z[C, h] = sum_c E[c, C] * (V/denom)[c, h]  # = E.T @ (V/denom)
    out[b] += z @ W_O[a].T

Sharding: 8 cores = 2 batches x 4 head-pairs. Each core handles one batch
and two heads; the attention matrix is device-local. Host sums the four
head-pair partial outputs per batch.

Device layouts (per core):
    xT16  [E, S]      x[b] transposed (host-side), fp16 (all projections)
    wk/wq/wv16 [E, 128]  W[a0].T | W[a1].T concat on head axis, fp16
    wo0/wo1 [128, E]  W_O[a].T duplicated on both partition halves
    maskb [128, 128]  additive causal triangle (0 or -1e9)
    maskm [128, 128]  multiplicative fp16 0/1 triangle (single-wave path)
    outT  [E, S]      partial output, transposed

On-chip flow per head: scores [c_blk=128 rows, 512-wide C chunks] are
matmul'd into rotating PSUM wave tiles (2 banks x 2 bufs); the diagonal
gets an additive -1e9 triangle mask; ScalarE applies exp (scale=1/sqrt(H))
writing the row panel to SBUF (fp16) with a fused per-row accumulation
(softmax denominator). z^T accumulates in PSUM across row blocks; chunk j
of C lives at partition half (j < NCH/2 ? 0 : 64) so z^T fits in 4 banks
and coexists with the wave tiles.

PE scheduling: the exp chain (ACT ~1.1us per 1024-wide wave) is slower
than the PE refill of a wave (~0.5us), so raw score streaming stalls PE
on PSUM recycling.  To keep PE dense, z-stage matmuls are interleaved
at CHUNK granularity: a global FIFO of ready z chunks (populated Z_LAG
row blocks late, once a block's denominator is known) is drained ZPW
chunks after every score/projection wave.  Head 1 sweeps rows in
reverse so z banks finalize early; bank-final events trigger the zsb
copy + output projection inline.  Each head's zT bank-zeroing matmuls
(start=True full-width zero writes, letting every real z chunk
accumulate start=False at its TRUE width — no padded columns) are
deferred to the head's first real z chunk so the head boundary
(head-0 drain -> zsb copy -> zero) rides under head-1's score waves
instead of stalling PE.  PSUM->SBUF copies stay on DVE (GPSIMD cannot
access PSUM on TRN2; and DVE is in-order, so long panel reduces must
NOT be queued there — they stall the latency-critical mask-adds).

NOTE on the clock: the HW power governor (HAM, k-of-n clock duty)
throttles the PE to half clock 55-67% of the time under this sustained
load, and the share drifts +-10pp run to run (~+-25us span).  Dense
scheduling does not unlock clock — only genuine MAC/energy reduction
moves the floor.  fp8 DoubleRow measured SLOWER than fp16 here and
fails accuracy; don't revisit.

Matmul dtypes: fp16 operands everywhere (PSUM accumulates fp32).  The
HAM clock governor throttles the PE under sustained dense activity, so
total PE column-cycles and operand power (not idle-gap avoidance) set
the floor; fp16 halves operand traffic vs f32r and keeps every matmul
at 1 col/cycle (f32r at 128-wide output runs at 1/4 rate).  Score
rounding from fp16 K/Q is ~1e-3 absolute on the exponent -> ~0.1% on
softmax weights.
"""

import math

import numpy as np

B, S_FULL, E, A, H = 2, 4096, 512, 8, 64
N_CORES = 8
NEG_BIG = -1.0e9

import os as _os

Z_LAG = int(_os.environ.get("Z_LAG", "3"))
ZPW = int(_os.environ.get("ZPW", "2"))
PANEL_BUFS = int(_os.environ.get("PANEL_BUFS", "4"))

_prog_cache = {}


def _build_program(S):
    import concourse.mybir as mybir
    import concourse.tile as tile
    from concourse import bacc

    f32 = mybir.dt.float32
    f32r = mybir.dt.float32r
    bf16 = mybir.dt.bfloat16
    fp16 = mybir.dt.float16

    EC = E // 128            # e chunks (contraction for projections)
    NCB = S // 128           # row blocks
    NCH = S // 512           # C chunks per full row
    HALF = NCH // 2          # chunks per partition half of z^T
    assert NCH % 2 == 0

    nc = bacc.Bacc("TRN2", target_bir_lowering=False, debug=False)

    xT16 = nc.dram_tensor("xT16", [E, S], fp16, kind="ExternalInput")
    # all small operands in one partition-major blob: 128 rows x
    # [wk(512) | wq(512) | wv(512) | wo01(512) | maskm(128)] fp16 --
    # 4 DMA transfers with 1-2KB-per-partition descriptors instead of
    # ~1500 256-byte ones that stall the x stream behind them.
    blob = nc.dram_tensor("blob", [128, 2176], fp16, kind="ExternalInput")
    maskb = nc.dram_tensor("maskb", [128, 128], f32, kind="ExternalInput")
    outT = nc.dram_tensor("outT", [E, S], fp16, kind="ExternalOutput")

    ExpF = mybir.ActivationFunctionType.Exp
    AxX = mybir.AxisListType.X
    AluAdd = mybir.AluOpType.add

    with tile.TileContext(nc) as tc:
        with (
            tc.tile_pool(name="singles", bufs=1) as singles,
            tc.tile_pool(name="panelp", bufs=PANEL_BUFS) as panelp,
            tc.tile_pool(name="small", bufs=8) as small,
            tc.tile_pool(name="outst", bufs=4) as outst,
            tc.tile_pool(name="ps", bufs=2, space="PSUM") as ps,
            tc.tile_pool(name="zps", bufs=1, space="PSUM") as zps,
        ):
            # ---- load inputs (small tensors first: weights + masks are on
            # early dependency chains; the bulk x stream follows) ----
            xt16 = singles.tile([128, EC, S], fp16)
            blb = singles.tile([128, 2176], fp16)
            wks = blb[:, 0:512]
            wqs = blb[:, 512:1024]
            wvs = blb[:, 1024:1536]
            wos01 = blb[:, 1536:2048]
            mskm = blb[:, 2048:2176]
            # z^T staging, packed for a single 128-contraction output
            # projection: partitions 0:64 = head 0, 64:128 = head 1;
            # zsbA holds chunk parity 0 (ccn < HALF), zsbB parity 1.
            zsbA = singles.tile([128, HALF * 512], fp16)
            zsbB = singles.tile([128, HALF * 512], fp16)
            msk = singles.tile([128, 128], f32)
            for bq in range(4):
                bsl = slice(bq * 544, (bq + 1) * 544)
                nc.sync.dma_start(out=blb[:, bsl], in_=blob[:, bsl])
            nc.sync.dma_start(out=msk, in_=maskb[:, :])
            SQ = S // 4
            for q in range(4):
                if q == 0:
                    # finer first slabs: the first kt/qt chunk needs only
                    # cols 0:512 of every e-chunk — land those first so the
                    # opening projection starts ~1/2 slab earlier
                    for half in range(2):
                        hsl = slice(half * 512, (half + 1) * 512)
                        for ec in range(EC):
                            nc.sync.dma_start(
                                out=xt16[:, ec, hsl],
                                in_=xT16[ec * 128:(ec + 1) * 128, hsl],
                            )
                    continue
                qsl = slice(q * SQ, (q + 1) * SQ)
                for ec in range(EC):
                    nc.sync.dma_start(
                        out=xt16[:, ec, qsl], in_=xT16[ec * 128:(ec + 1) * 128, qsl]
                    )
            # bf16 always: the K=1 zeroing matmuls are invalid ISA in f32r,
            # and mixing dtypes across an accumulation group is fine.
            zero_t = singles.tile([1, 576], bf16)
            nc.vector.memset(zero_t, 0.0)

            kt = singles.tile([128, S], fp16)
            qt = singles.tile([128, S], fp16)
            vsb = singles.tile([128, NCB, 128], fp16)
            CPQ = NCH // 4           # 512-chunks per x quarter

            # ---- global z-chunk FIFO -------------------------------------
            # Items: (ensure_pre, zT, vt, panel, j, poff, col, start, stop,
            #         post) — ensure_pre emits the head's deferred zero
            # matmuls; post (on an item's final chunk) runs bank-final
            # hooks (zsb copy + output projection).
            zq = []

            def pump(k):
                for _ in range(k):
                    if not zq:
                        return
                    ensure, zT_, vt_, panel_, j, poff, col, w_n, sp_, post = zq.pop(0)
                    if ensure is not None:
                        ensure()
                    if vt_ is not None:
                        nc.tensor.matmul(
                            zT_[poff:poff + 64, col:col + w_n],
                            vt_,
                            panel_[:, j * 512:j * 512 + w_n],
                            start=False, stop=sp_,
                            skip_group_check=True,
                        )
                    if post is not None:
                        post()

            # ---- projections (emitted per x-quarter, interleaved into the
            # head-0 narrow blocks so PE stays dense while DMA streams in) ----
            def emit_proj(q):
                for dst, w in ((kt, wks), (qt, wqs)):
                    for cc in range(q * CPQ, (q + 1) * CPQ):
                        csl = slice(cc * 512, (cc + 1) * 512)
                        wt = ps.tile([128, 1024], f32, tag="wave", name="wt")
                        for ec in range(EC):
                            nc.tensor.matmul(
                                wt[:, :512], w[:, ec * 128:(ec + 1) * 128],
                                xt16[:, ec, csl],
                                start=(ec == 0), stop=(ec == EC - 1),
                            )
                        nc.vector.tensor_copy(dst[:, csl], wt[:, :512])
                        pump(ZPW)
                for cb in range(q * (NCB // 4), (q + 1) * (NCB // 4)):
                    bsl = slice(cb * 128, (cb + 1) * 128)
                    wt = ps.tile([128, 1024], f32, tag="wave", name="wt")
                    for ec in range(EC):
                        nc.tensor.matmul(
                            wt[:, :128], xt16[:, ec, bsl],
                            wvs[:, ec * 128:(ec + 1) * 128],
                            start=(ec == 0), stop=(ec == EC - 1),
                        )
                    nc.vector.tensor_copy(vsb[:, cb, :], wt[:, :128])
                    pump(1)

            # ---- attention per head ----
            for h in range(2):
                hs = slice(h * 64, (h + 1) * 64)
                # head 0 forward, head 1 reverse: the reverse head leads with
                # its widest blocks and finalizes z banks early
                order = list(range(NCB - 1, -1, -1)) if h == 1 else list(range(NCB))
                zT = zps.tile([128, HALF * 512], f32, name="zT")

                zero_done = [False]

                def ensure_zero(zT=zT, zero_done=zero_done):
                    # Dummy matmuls zero-start BOTH partition halves of each
                    # z bank; every real z chunk then accumulates with
                    # start=False at its true width (no 512-padding MACs for
                    # diagonal chunks).  Deferred to the head's first z chunk
                    # so the head boundary doesn't stall PE.
                    if not zero_done[0]:
                        zero_done[0] = True
                        for k in range(HALF):
                            for poff in (0, 64):
                                nc.tensor.matmul(
                                    zT[poff:poff + 64, k * 512:(k + 1) * 512],
                                    zero_t[:, :64], zero_t[:, 64:576],
                                    start=True, stop=False,
                                    skip_group_check=True,
                                )

                first_cb = {}
                last_cb = {}
                for j in range(NCH):
                    part = [cb for cb in order if cb >= 4 * j]
                    first_cb[j] = part[0]
                    last_cb[j] = part[-1]

                def emit_out_proj(ccn):
                    zsb = zsbA if ccn < HALF else zsbB
                    col = (ccn % HALF) * 512
                    for ecn in range(EC):
                        esl = slice(ecn * 128, (ecn + 1) * 128)
                        wt = ps.tile([128, 1024], f32, tag="wave", name="wt")
                        nc.tensor.matmul(
                            wt[:, :512],
                            wos01[:, esl],
                            zsb[:, col:col + 512],
                            start=True, stop=True,
                        )
                        st = outst.tile([128, 512], fp16, name="st")
                        nc.vector.tensor_copy(st, wt[:, :512])
                        nc.sync.dma_start(
                            out=outT[esl, ccn * 512:(ccn + 1) * 512], in_=st
                        )

                def push_z(item, h=h, zT=zT, ensure_zero=ensure_zero,
                           first_cb=first_cb, last_cb=last_cb):
                    vt_i, panel_i, nch_i, cb_i = item
                    cw = cb_i * 128 + 128   # true row length of this block
                    post = None
                    if h == 1 and cb_i % 4 == 0 and cb_i // 4 < HALF:
                        # Reverse sweep: bank q (chunks q, q+HALF) is final
                        # once cb=4q is done; copy it out and trickle the
                        # output projection into the PE stream right here.
                        q = cb_i // 4

                        def post(q=q, zT=zT, h=h):
                            csl = slice(q * 512, (q + 1) * 512)
                            nc.vector.tensor_copy(
                                zsbA[64 * h:64 * h + 64, csl], zT[0:64, csl]
                            )
                            nc.vector.tensor_copy(
                                zsbB[64 * h:64 * h + 64, csl], zT[64:128, csl]
                            )
                            emit_out_proj(q)
                            emit_out_proj(q + HALF)

                    for j in range(nch_i):
                        poff = 0 if j < HALF else 64
                        col = (j % HALF) * 512
                        w_n = min(cw - j * 512, 512)
                        stop = cb_i == last_cb[j]
                        zq.append((
                            ensure_zero, zT, vt_i, panel_i, j, poff, col,
                            w_n, stop,
                            post if j == nch_i - 1 else None,
                        ))

                pending = []
                GRP = NCB // 4
                for oi, cb in enumerate(order):
                    if h == 0 and oi % GRP == 0:
                        emit_proj(oi // GRP)
                    c0 = cb * 128
                    nch = (c0 + 128 + 511) // 512
                    nwaves = (nch + 1) // 2
                    lastw = c0 + 128 - (nch - 1) * 512   # width of diag chunk
                    panel = panelp.tile([128, S], fp16, name="panel")
                    rsp = small.tile([128, 4], f32, name="rsp")
                    # Multi-wave blocks keep the ACT fused accumulation for
                    # the denominator: DVE is in-order and carries the
                    # latency-critical PSUM mask-adds + wave copies, so
                    # multi-microsecond panel reduces there stall the PSUM
                    # recycle chain (cost ~25us when tried).
                    red_eng = None if nwaves == 1 else "act"
                    for wv_i in range(nwaves):
                        jlo = 2 * wv_i
                        jhi = min(jlo + 2, nch)
                        wt = ps.tile([128, 1024], f32, tag="wave", name="wt")
                        for j in range(jlo, jhi):
                            w_n = lastw if j == nch - 1 else 512
                            nc.tensor.matmul(
                                wt[:, (j - jlo) * 512:(j - jlo) * 512 + w_n],
                                kt[hs, c0:c0 + 128],
                                qt[hs, j * 512:j * 512 + w_n],
                                start=True, stop=True,
                            )
                        if jhi == nch and nwaves > 1:
                            # mask only the last 128 cols (the true triangle);
                            # earlier diag-chunk cols are fully valid
                            o = c0 - (nch - 1) * 512
                            dlo = (nch - 1 - jlo) * 512 + o
                            nc.vector.tensor_add(
                                wt[:, dlo:dlo + 128], wt[:, dlo:dlo + 128],
                                msk,
                            )
                        wlen = (jhi - jlo - 1) * 512 + (lastw if jhi == nch else 512)
                        nc.scalar.activation(
                            out=panel[:, jlo * 512:jlo * 512 + wlen],
                            in_=wt[:, :wlen],
                            func=ExpF,
                            scale=1.0 / math.sqrt(H),
                            accum_out=(rsp[:, wv_i:wv_i + 1]
                                       if red_eng == "act" else None),
                        )
                        pump(ZPW)
                    den = small.tile([128, 1], f32, name="den")
                    if red_eng == "act":
                        nc.vector.tensor_reduce(den, rsp[:, :nwaves], axis=AxX, op=AluAdd)
                    else:
                        # single-wave: mask applied post-exp (0/1 triangle on
                        # the fp16 panel) so exp never waits the mask; the
                        # reduce then sums the masked zeros
                        dpan = (nch - 1) * 512 + (c0 - (nch - 1) * 512)
                        nc.vector.tensor_mul(
                            panel[:, dpan:dpan + 128],
                            panel[:, dpan:dpan + 128], mskm,
                        )
                        nc.vector.tensor_reduce(
                            den, panel[:, :c0 + 128], axis=AxX, op=AluAdd
                        )
                    rden = small.tile([128, 1], f32, name="rden")
                    nc.vector.reciprocal(rden, den)
                    vt = small.tile([128, 64], fp16, name="vt")
                    nc.vector.tensor_scalar_mul(vt, vsb[:, cb, hs], rden)
                    pending.append((vt, panel, nch, cb))
                    if len(pending) > Z_LAG:
                        push_z(pending.pop(0))
                # Spill this head's remaining blocks into the FIFO; head 0's
                # tail rides under head 1's first (widest) score waves.
                for item in pending:
                    push_z(item)
                if h == 0:
                    def h0_copies(zT=zT):
                        for q in range(HALF):
                            csl = slice(q * 512, (q + 1) * 512)
                            nc.vector.tensor_copy(
                                zsbA[0:64, csl], zT[0:64, csl]
                            )
                            nc.vector.tensor_copy(
                                zsbB[0:64, csl], zT[64:128, csl]
                            )
                    # sentinel: runs after head 0's last z chunk pops
                    zq.append((None, None, None, None, 0, 0, 0, 0, False,
                               h0_copies))
            # drain
            pump(len(zq) + 8)

    nc.compile()
    return nc


def get_program(S=S_FULL):
    if S not in _prog_cache:
        _prog_cache[S] = _build_program(S)
    return _prog_cache[S]


def make_mask_band():
    """Triangle mask for the last 128 cols of a diagonal chunk:
    col t (relative to the diagonal start) is valid iff t <= r."""
    r = np.arange(128)[:, None]
    t = np.arange(128)[None, :]
    return np.where(t <= r, 0.0, NEG_BIG).astype(np.float32)


def make_core_inputs(x, W_K, W_Q, W_V, W_O, core):
    """Inputs for core = b*4 + g (batch b, head pair a0=2g, a1=2g+1)."""
    b, g = divmod(core, 4)
    a0, a1 = 2 * g, 2 * g + 1
    xT = np.ascontiguousarray(x[b].T)
    wk = np.ascontiguousarray(np.concatenate([W_K[a0].T, W_K[a1].T], axis=1))
    wq = np.ascontiguousarray(np.concatenate([W_Q[a0].T, W_Q[a1].T], axis=1))
    wv = np.ascontiguousarray(np.concatenate([W_V[a0].T, W_V[a1].T], axis=1))
    wo01 = np.ascontiguousarray(np.concatenate([W_O[a0].T, W_O[a1].T], axis=0))
    tri = make_mask_band()
    f16 = np.float16

    def fold(w):
        # [E, 128] -> [128, EC*128]: SBUF partition p holds w[ec*128+p, :]
        # at cols ec*128..ec*128+128
        return w.reshape(4, 128, 128).transpose(1, 0, 2).reshape(128, 512)

    blob = np.hstack([
        fold(wk), fold(wq), fold(wv), wo01, (tri == 0.0).astype(np.float32),
    ]).astype(f16)
    return {
        "xT16": xT.astype(f16),
        "blob": np.ascontiguousarray(blob),
        "maskb": tri,
    }


def run_on_cores(inputs, trace=False):
    from concourse.bass_utils import run_bass_kernel_spmd

    nc = get_program()
    in_maps = [
        make_core_inputs(
            inputs["x"], inputs["W_K"], inputs["W_Q"], inputs["W_V"],
            inputs["W_O"], core,
        )
        for core in range(N_CORES)
    ]
    return run_bass_kernel_spmd(
        nc, in_maps, list(range(N_CORES)), trace=trace,
    )


def kernel(x, W_K, W_Q, W_V, W_O):
    x = np.asarray(x, dtype=np.float32)
    W_K = np.asarray(W_K, dtype=np.float32)
    W_Q = np.asarray(W_Q, dtype=np.float32)
    W_V = np.asarray(W_V, dtype=np.float32)
    W_O = np.asarray(W_O, dtype=np.float32)
    res = run_on_cores(
        {"x": x, "W_K": W_K, "W_Q": W_Q, "W_V": W_V, "W_O": W_O}
    )
    out = np.zeros((B, S_FULL, E), dtype=np.float32)
    for b in range(B):
        acc = np.zeros((E, S_FULL), dtype=np.float32)
        for g in range(4):
            acc += res.results[b * 4 + g]["outT"]
        out[b] = acc.T
    return out
